# revision 6
# baseline (speedup 1.0000x reference)
"""Trainium2 Bass kernel for nn_DecoderRNN (teacher-forced LSTM decoder).

Sharding: pure data-parallel over batch. Each of the 8 NeuronCores gets 8
sequences and runs the full pipeline for them:
  phase A: XW = relu(emb[tok]) @ W_ih.T + (b_ih + b_hh)     (batched matmul)
  phase B: 60-step LSTM recurrence (h @ W_hh.T streamed through the PE with
           4-way column-group tiling, since the per-core batch M=8 is thin)
  phase C: logits = hs @ W_out.T + b_out ; log_softmax ; write [8,60,V] f32

No collectives. Host side only does input sharding / layout prep (gather of
embedding rows, transposes, dtype casts) and output concatenation.
"""

import sys

import numpy as np

for _p in ("/opt/trn_rl_repo",):
    if _p not in sys.path:
        sys.path.append(_p)

import ml_dtypes  # noqa: E402
from concourse import bacc, mybir, tile  # noqa: E402
from concourse.bass_utils import run_bass_kernel_spmd  # noqa: E402
from concourse.masks import make_identity  # noqa: E402

F32 = mybir.dt.float32
BF16 = mybir.dt.bfloat16
FP8 = mybir.dt.float8e4
NPBF16 = ml_dtypes.bfloat16

B, T, H, V = 64, 60, 1024, 32000
NCORES = 8
BL = B // NCORES            # 8 sequences per core
NTOK = T * BL               # 480 tokens per core, tok = t*BL + b
KC = H // 128               # 8 contraction chunks
GATE_PERM = [0, 2, 1, 3]    # (i,f,g,o) -> (i,g,f,o): {i,g} in cols 0:512
MS = [128, 128, 128, 96]    # token-tile sizes (480 = 3*128 + 96)
NV = 63                     # vocab tiles: 62*512 + 256
VS = [512] * 62 + [256]

_PROG = None  # cached compiled program


def _build_program():
    nc = bacc.Bacc("TRN2", target_bir_lowering=False, debug=False,
                   num_devices=NCORES)
    d = {}
    d["xT"] = nc.dram_tensor("xT", [H, NTOK], F32, kind="ExternalInput").ap()
    d["wih"] = nc.dram_tensor("wih", [H, 4 * H], F32, kind="ExternalInput").ap()
    d["bsum"] = nc.dram_tensor("bsum", [1, 4 * H], F32, kind="ExternalInput").ap()
    d["whh"] = nc.dram_tensor("whh", [H, 4 * H], BF16, kind="ExternalInput").ap()
    d["hT0"] = nc.dram_tensor("hT0", [128, KC * BL], BF16, kind="ExternalInput").ap()
    d["c0q"] = nc.dram_tensor("c0q", [128, 256], F32, kind="ExternalInput").ap()
    d["woutT"] = nc.dram_tensor("woutT", [H, V], BF16, kind="ExternalInput").ap()
    d["bout"] = nc.dram_tensor("bout", [1, V], BF16, kind="ExternalInput").ap()
    d["out_lp"] = nc.dram_tensor("out_lp", [BL, T, V], F32, kind="ExternalOutput").ap()
    d["h_T"] = nc.dram_tensor("h_T", [BL, H], F32, kind="ExternalOutput").ap()
    d["c_T"] = nc.dram_tensor("c_T", [BL, H], F32, kind="ExternalOutput").ap()

    with tile.TileContext(nc) as tc:
        _body(tc, d)
    nc.compile()
    return nc


def _body(tc, d):
    nc = tc.nc
    Sig = mybir.ActivationFunctionType.Sigmoid
    Tanh = mybir.ActivationFunctionType.Tanh
    Exp = mybir.ActivationFunctionType.Exp
    Ln = mybir.ActivationFunctionType.Ln

    with tc.tile_pool(name="persist", bufs=1) as pp:
        hsT = pp.tile([128, KC * NTOK], BF16, tag="hsT")
        sume = pp.tile([128, 4 * NV], F32, tag="sume")
        ident = pp.tile([128, 128], F32, tag="ident")
        ones_x = pp.tile([1, 128], F32, tag="ones_x")
        ones_c = pp.tile([1, 128], BF16, tag="ones_c")
        hT0_sb = pp.tile([128, KC * BL], BF16, tag="hT0")
        make_identity(nc, ident[:])
        nc.vector.memset(ones_x[:], 1.0)
        nc.vector.memset(ones_c[:], 1.0)
        nc.sync.dma_start(hT0_sb[:], d["hT0"][:])

        with tc.tile_pool(name="ab", bufs=1) as pab:
            XW = [pab.tile([128, 4 * H], F32, tag=f"xw{m}", name=f"xw{m}") for m in range(4)]

            # ---------------- phase A: XW = xT.T @ W_ihT + b ----------------
            with (
                tc.tile_pool(name="pa", bufs=1) as pa,
                tc.tile_pool(name="paw", bufs=16) as paw,
                tc.tile_pool(name="psA", bufs=4, space="PSUM") as psA,
            ):
                xT_sb = [pa.tile([128, NTOK], F32, tag=f"xt{k}", name=f"xt{k}") for k in range(KC)]
                for k in range(KC):
                    nc.sync.dma_start(xT_sb[k][:], d["xT"][k * 128:(k + 1) * 128, :])
                for n in range(8):
                    wts = [paw.tile([128, 512], F32, tag="wih", name="wihT") for _ in range(KC)]
                    bsum_t = paw.tile([1, 512], F32, tag="bsum")
                    nc.sync.dma_start(bsum_t[:], d["bsum"][:, n * 512:(n + 1) * 512])
                    for k in range(KC):
                        nc.sync.dma_start(
                            wts[k][:],
                            d["wih"][k * 128:(k + 1) * 128, n * 512:(n + 1) * 512])
                    for m in range(4):
                        ms = MS[m]
                        ps = psA.tile([128, 512], F32, tag="psA")
                        for k in range(KC):
                            nc.tensor.matmul(
                                ps[:ms], xT_sb[k][:, m * 128:m * 128 + ms], wts[k][:],
                                start=(k == 0), stop=False)
                        nc.tensor.matmul(
                            ps[:ms], ones_x[:, :ms],
                            bsum_t[:],
                            start=False, stop=True)
                        nc.vector.tensor_copy(
                            XW[m][:ms, n * 512:(n + 1) * 512], ps[:ms])

            # ---------------- phase B: LSTM recurrence ----------------------
            with (
                tc.tile_pool(name="pb", bufs=1) as pb,
                tc.tile_pool(name="pbs", bufs=2) as pbs,
                tc.tile_pool(name="psB", bufs=1, space="PSUM") as psB,
                tc.tile_pool(name="psT", bufs=2, space="PSUM") as psT,
            ):
                whh_sb = [pb.tile([128, 4 * H], BF16, tag=f"whh{k}", name=f"whh{k}") for k in range(KC)]
                for k in range(KC):
                    nc.sync.dma_start(whh_sb[k][:], d["whh"][k * 128:(k + 1) * 128, :])
                c_prev = pbs.tile([128, 256], F32, tag="c")
                nc.sync.dma_start(c_prev[:], d["c0q"][:])
                # persistent ping-pong buffers, memset once so that the
                # partitions not covered by the 4x8 batch groups hold zeros
                # (they are computed on but never read as real data)
                xw_bufs = [pb.tile([128, 1024], F32, tag=f"xwb{i}",
                                   name=f"xwb{i}") for i in range(2)]
                gates_bufs = [psB.tile([128, 1024], F32, tag=f"gps{i}",
                                       name=f"gps{i}") for i in range(2)]
                for i in range(2):
                    nc.vector.memset(xw_bufs[i][:], 0.0)
                    nc.vector.memset(gates_bufs[i][:], 0.0)

                h_q = None
                for t in range(T):
                    m, p0 = t // 16, (t % 16) * 8
                    # xw_t: [128,1024] quarter layout from XW[m]
                    xw = xw_bufs[t % 2]
                    for q in range(4):
                        src = XW[m][p0:p0 + 8, :].rearrange(
                            "p (g q j) -> p g q j", g=4, q=4)[:, :, q, :]
                        dst = xw[32 * q:32 * q + 8, :].rearrange(
                            "p (g j) -> p g j", g=4)
                        nc.sync.dma_start(dst, src)

                    gates = gates_bufs[t % 2]
                    for nh in range(2):
                        for k in range(KC):
                            if t == 0:
                                lhsT = hT0_sb[:, k * BL:(k + 1) * BL]
                            else:
                                off = k * NTOK + (t - 1) * BL
                                lhsT = hsT[:, off:off + BL]
                            for q in range(4):
                                rhs = whh_sb[k][:, q * 1024 + nh * 512:
                                                q * 1024 + (nh + 1) * 512]
                                nc.tensor.matmul(
                                    gates[32 * q:32 * q + 8,
                                          nh * 512:(nh + 1) * 512],
                                    lhsT, rhs,
                                    start=(k == 0), stop=(k == KC - 1),
                                    tile_position=(0, 32 * q))
                    gsb = pbs.tile([128, 1024], F32, tag="gsb")
                    # cols: 0:256 = i, 256:512 = g, 512:768 = f, 768:1024 = o
                    nc.vector.tensor_add(gsb[:, 0:512], gates[:, 0:512],
                                         xw[:, 0:512])
                    nc.vector.tensor_add(gsb[:, 512:1024], gates[:, 512:1024],
                                         xw[:, 512:1024])
                    ig = pbs.tile([128, 256], F32, tag="ig")
                    gg = pbs.tile([128, 256], F32, tag="gg")
                    fo = pbs.tile([128, 512], F32, tag="fo")
                    nc.scalar.activation(ig[:], gsb[:, 0:256], Sig)
                    nc.scalar.activation(gg[:], gsb[:, 256:512], Tanh)
                    nc.scalar.activation(fo[:], gsb[:, 512:1024], Sig)
                    t1 = pbs.tile([128, 256], F32, tag="t1")
                    nc.vector.tensor_mul(t1[:], ig[:], gg[:])
                    t2 = pbs.tile([128, 256], F32, tag="t2")
                    nc.vector.tensor_mul(t2[:], fo[:, 0:256], c_prev[:])
                    c_new = pbs.tile([128, 256], F32, tag="c")
                    nc.vector.tensor_add(c_new[:], t1[:], t2[:])
                    th = pbs.tile([128, 256], F32, tag="th")
                    nc.scalar.activation(th[:], c_new[:], Tanh)
                    h_q = pbs.tile([128, 256], F32, tag="hq")
                    nc.vector.tensor_mul(h_q[:], fo[:, 256:512], th[:])
                    # transpose h -> hsT slots
                    tp = psT.tile([128, 256], F32, tag="tp")
                    for half in range(2):
                        nc.tensor.transpose(
                            tp[:, half * 128:(half + 1) * 128],
                            h_q[:, half * 128:(half + 1) * 128], ident[:])
                        src = tp[:, half * 128:(half + 1) * 128].rearrange(
                            "p (q c) -> p q c", q=4)[:, :, 0:8]
                        dst = hsT[:].rearrange(
                            "p (k s) -> p k s", k=KC)[:, half::2,
                                                      t * BL:(t + 1) * BL]
                        nc.vector.tensor_copy(dst, src)
                    c_prev = c_new

                # final h/c outputs (layout [32q+b, j'] -> [b, q*256+j'])
                for q in range(4):
                    nc.sync.dma_start(d["h_T"][:, q * 256:(q + 1) * 256],
                                      h_q[32 * q:32 * q + 8, :])
                    nc.sync.dma_start(d["c_T"][:, q * 256:(q + 1) * 256],
                                      c_prev[32 * q:32 * q + 8, :])

        # ---------------- phase C: projection + log_softmax -----------------
        with (
            tc.tile_pool(name="pc", bufs=1) as pc,
            tc.tile_pool(name="pcw", bufs=18) as pcw,
            tc.tile_pool(name="pcs", bufs=3) as pcs,
            tc.tile_pool(name="psC", bufs=4, space="PSUM") as psC,
        ):
            lg8 = pc.tile([128, 4 * V], FP8, tag="lg8")
            for v in range(NV):
                vs = VS[v]
                wv = [pcw.tile([128, 512], BF16, tag="wout", name="woutT") for _ in range(KC)]
                bout_t = pcw.tile([1, 512], BF16, tag="bout")
                nc.sync.dma_start(bout_t[:, :vs], d["bout"][:, v * 512:v * 512 + vs])
                for k in range(KC):
                    nc.sync.dma_start(
                        wv[k][:, :vs],
                        d["woutT"][k * 128:(k + 1) * 128, v * 512:v * 512 + vs])
                for m in range(4):
                    ms = MS[m]
                    ps = psC.tile([128, 512], F32, tag="psC")
                    for k in range(KC):
                        off = k * NTOK + m * 128
                        nc.tensor.matmul(ps[:ms, :vs], hsT[:, off:off + ms],
                                         wv[k][:, :vs],
                                         start=(k == 0), stop=False)
                    nc.tensor.matmul(ps[:ms, :vs], ones_c[:, :ms],
                                     bout_t[:, :vs],
                                     start=False, stop=True)
                    ex = pcs.tile([128, 512], BF16, tag="ex")
                    nc.scalar.activation(
                        ex[:ms, :vs], ps[:ms, :vs], Exp,
                        accum_out=sume[:ms, m * NV + v:m * NV + v + 1])
                    nc.vector.tensor_copy(
                        lg8[:ms, m * V + v * 512:m * V + v * 512 + vs],
                        ps[:ms, :vs])
            # pass 2: lse + subtract + write out
            for m in range(4):
                ms = MS[m]
                ntp = ms // 8
                s = pc.tile([128, 1], F32, tag=f"s{m}")
                nc.vector.tensor_reduce(
                    s[:ms], sume[:ms, m * NV:(m + 1) * NV],
                    axis=mybir.AxisListType.X, op=mybir.AluOpType.add)
                lse = pc.tile([128, 1], F32, tag=f"lse{m}")
                nc.scalar.activation(lse[:ms], s[:ms], Ln)
                for v in range(NV):
                    vs = VS[v]
                    st = pcs.tile([128, 512], F32, tag="st")
                    nc.vector.tensor_scalar(
                        st[:ms, :vs],
                        lg8[:ms, m * V + v * 512:m * V + v * 512 + vs],
                        lse[:ms], None, op0=mybir.AluOpType.subtract)
                    dst = d["out_lp"][:, m * 16:m * 16 + ntp,
                                      v * 512:v * 512 + vs].rearrange(
                                          "b t v -> t b v")
                    nc.sync.dma_start(dst, st[:ms, :vs])


# --------------------------- host-side prep ---------------------------------

def _core_inputs(k, tokens, emb, wihT_s, bsum_s, whh_s, h0, c0, woutT, bout):
    tok_k = tokens[k * BL:(k + 1) * BL]                     # [8, 60]
    x = emb[tok_k]                                          # [8, 60, H] f32
    np.maximum(x, 0.0, out=x)
    x_t = np.ascontiguousarray(x.transpose(1, 0, 2)).reshape(NTOK, H)
    xT = np.ascontiguousarray(x_t.T)                        # [H, 480]

    h0k = h0[0, k * BL:(k + 1) * BL]                        # [8, H]
    hT0 = np.ascontiguousarray(
        h0k.reshape(BL, KC, 128).transpose(2, 1, 0)).reshape(128, KC * BL)

    c0k = c0[0, k * BL:(k + 1) * BL]                        # [8, H]
    c0q = np.zeros((4, 32, 256), np.float32)
    c0q[:, :BL] = c0k.reshape(BL, 4, 256).transpose(1, 0, 2)
    c0q = c0q.reshape(128, 256)

    return {
        "xT": xT.astype(np.float32),
        "wih": wihT_s,
        "bsum": bsum_s,
        "whh": whh_s,
        "hT0": hT0.astype(NPBF16),
        "c0q": c0q,
        "woutT": woutT,
        "bout": bout,
    }


def prep_all_inputs(encoder_outputs, h0, c0, target_tensor, emb, W_ih, W_hh,
                    b_ih, b_hh, W_out, b_out):
    h0 = np.asarray(h0, np.float32)
    c0 = np.asarray(c0, np.float32)
    tgt = np.asarray(target_tensor)
    emb = np.asarray(emb, np.float32)
    W_ih = np.asarray(W_ih, np.float32)
    W_hh = np.asarray(W_hh, np.float32)
    b_ih = np.asarray(b_ih, np.float32)
    b_hh = np.asarray(b_hh, np.float32)
    W_out = np.asarray(W_out, np.float32)
    b_out = np.asarray(b_out, np.float32)

    tokens = np.concatenate(
        [np.zeros((B, 1), np.int64), tgt[:, :T - 1]], axis=1).astype(np.int64)

    # W_ih.T with output cols permuted to gate order (i,g,f,o)
    wihT_s = np.ascontiguousarray(
        W_ih.reshape(4, H, H)[GATE_PERM].reshape(4 * H, H).T)
    bsum_s = (b_ih + b_hh).reshape(4, H)[GATE_PERM].reshape(1, 4 * H).copy()
    # W_hh.T in stream layout: col = q*1024 + gate'*256 + j'
    whh_s = np.ascontiguousarray(
        W_hh.T.reshape(H, 4, 4, 256)[:, GATE_PERM].transpose(0, 2, 1, 3)
        .reshape(H, 4 * H)).astype(NPBF16)
    woutT = np.ascontiguousarray(W_out.T).astype(NPBF16)
    boutr = b_out.reshape(1, V).astype(NPBF16)

    return [
        _core_inputs(k, tokens, emb, wihT_s, bsum_s, whh_s, h0, c0, woutT,
                     boutr)
        for k in range(NCORES)
    ]


def assemble_outputs(results):
    log_probs = np.concatenate([r["out_lp"] for r in results], axis=0)
    h_T = np.concatenate([r["h_T"] for r in results], axis=0)[None]
    c_T = np.concatenate([r["c_T"] for r in results], axis=0)[None]
    return log_probs, h_T, c_T


def get_program():
    global _PROG
    if _PROG is None:
        _PROG = _build_program()
    return _PROG


def kernel(**inputs):
    nc = get_program()
    in_maps = prep_all_inputs(**inputs)
    res = run_bass_kernel_spmd(nc, in_maps, list(range(NCORES)))
    return assemble_outputs(res.results)
